# revision 9
# baseline (speedup 1.0000x reference)
"""Trainium2 Bass kernel for nn_EquiLinearLayer.

Computes  out[s,n,j,y] = sum_{i,x,b} weights[j,i,b] * blade[b,x,y] * x[s,n,i,x]
for x:[8,2048,512,16] f32, weights:[512,512,9] f32, blade:[9,16,16] f32.

Strategy (all-TensorE, two matmul phases, data-parallel over points):
  * 16384 points are split across 8 cores (2048 each), grouped in 8s
    (dpt in 0..7), 256 groups per core.
  * Host preps:
      xT[g, (dpt,x), i]           [G,128,512]  per-core slice of x
      RB[(dpt,x), b*128+(dpt,y)]  [128,1280]   block-diag replicated blade
      W2[b, i, j]                 [9,512,512]  transposed weights
  * Phase 1 (per group, per i-chunk ic): one matmul per 512-col quad of RB:
      xbt[ic][i_loc, b*128+(dpt,y)] = sum_{(dpt,x)} xT[g,(dpt,x),ic*128+i_loc]
                                                    * RB[(dpt,x), col]
    which equals xb[pt,i,b,y] = sum_x x[pt,i,x]*blade[b,x,y].
  * Phase 2 (per group): contraction over (i,b) into one PSUM bank:
      out2[(dpt,y), j] += xbt[ic][:, b*128:(b+1)*128].T @ W2[b, ic-chunk, :]
    = out[pt, j, y].
    The first NB_FP8 b-blocks run as fp8e4 DoubleRow matmuls (2 i-chunks per
    matmul, ~1.5x TensorE rate); the rest run in float32r. For the fp8 part
    the weights are shifted by -0.5 before e4m3 quantization (halves the
    W-side quantization error since weights ~ U[0,1)); the exact correction
      0.5 * sum_{i, b<NB_FP8} xb[pt,i,b,y]
        = sum_x (sum_i x[pt,i,x]) * (0.5 * sum_{b<NB_FP8} blade[b,x,y])
    is computed on the host (a [128]-vector per 8-point group, ~0.1% extra
    DMA) and added as a per-partition bias at PSUM evac on the Act engine.
  * Host reorders the b axis so the fp8-assigned blades are the ones with the
    smallest signal energy (least added quantization error).
  * fp32r matmuls run at full PE rate (plain fp32 is 4x slower).
"""

from contextlib import ExitStack

import numpy as np
import ml_dtypes

import concourse.bass as bass
import concourse.mybir as mybir
import concourse.tile as tile
from concourse import bacc
from concourse.bass_utils import run_bass_kernel_spmd

BATCH, NPTS, C, MV, BL = 8, 2048, 512, 16, 9
J = 512
N_CORES = 8
TOTAL_PTS = BATCH * NPTS            # 16384
GROUPS = TOTAL_PTS // 8             # 2048 groups of 8 points
GROUPS_PER_CORE = GROUPS // N_CORES  # 256

F32 = mybir.dt.float32
FP8 = mybir.dt.float8e4
E4M3 = ml_dtypes.float8_e4m3
QUADS = ((0, 512), (512, 512), (1024, 256))  # RB column quads (last is padded)

NB_FP8 = 9          # how many of the 9 b-blocks run in fp8 DoubleRow
SCOL = 1152         # S-correction column inside the phase-1 PSUM tile


def build_program(groups: int = GROUPS_PER_CORE, nb: int = NB_FP8,
                  repeats: int = 1) -> bass.Bass:
    nc = bacc.Bacc(trn_type="TRN2", target_bir_lowering=False, debug=False)
    mmdt = mybir.dt.float32r
    DR = mybir.MatmulPerfMode.DoubleRow
    nfull = BL - nb                  # b-blocks kept in fp32r

    xT_d = nc.dram_tensor("xT", [groups, 128, C], mmdt, kind="ExternalInput")
    rb_d = nc.dram_tensor("RB", [128, 1280], mmdt, kind="ExternalInput")
    out_d = nc.dram_tensor("outT", [groups, 128, J], F32, kind="ExternalOutput")
    if nb:
        w8_d = nc.dram_tensor("W8", [2 * nb, 128, 2, J], FP8, kind="ExternalInput")
        s_d = nc.dram_tensor("SG", [groups, 128, 1], F32, kind="ExternalInput")
    if nfull:
        w2_d = nc.dram_tensor("W2", [nfull, 4, 128, J], mmdt, kind="ExternalInput")

    with tile.TileContext(nc) as tc, ExitStack() as ctx:
        const = ctx.enter_context(tc.tile_pool(name="const", bufs=1))
        xtp = ctx.enter_context(tc.tile_pool(name="xtp", bufs=4))
        xbp = ctx.enter_context(tc.tile_pool(name="xbp", bufs=3))
        ssb = ctx.enter_context(tc.tile_pool(name="ssb", bufs=3))
        osb = ctx.enter_context(tc.tile_pool(name="osb", bufs=4))
        ps1 = ctx.enter_context(tc.tile_pool(name="ps1", bufs=2, space="PSUM"))
        ps2 = ctx.enter_context(tc.tile_pool(name="ps2", bufs=2, space="PSUM"))

        w8t, w2t = {}, {}
        for q in range(2 * nb):
            t = const.tile([128, 2, J], FP8, tag=f"w8_{q}")
            nc.sync.dma_start(out=t[:], in_=w8_d[q])
            w8t[q] = t
        for bi in range(nfull):
            for ic in range(4):
                t = const.tile([128, J], mmdt, tag=f"w2_{bi}_{ic}")
                nc.sync.dma_start(out=t[:], in_=w2_d[bi, ic])
                w2t[bi, ic] = t
        rbt = const.tile([128, 1280], mmdt, tag="rb")
        nc.sync.dma_start(out=rbt[:], in_=rb_d[:])

        def ph1_half(g, xt, ics, xb8, xb32):
            """Phase-1 matmuls + psum evac for the given i-chunks.

            Evac engines: DVE for the first half (ics 0,1), Act for the
            second (GPSIMD/Pool cannot read PSUM), so the two halves' copies
            run concurrently.
            """
            eng = nc.vector if ics[0] == 0 else nc.scalar
            for ic in ics:
                p1 = ps1.tile([128, 1280], F32, tag="p1")
                for c0, n in QUADS:
                    nc.tensor.matmul(
                        p1[:, c0:c0 + n],
                        xt[:, ic * 128:(ic + 1) * 128],
                        rbt[:, c0:c0 + n],
                        start=True, stop=True,
                    )
                if nb:
                    # fp8 b-blocks 0..nb-1 -> DoubleRow k-tile pairs.
                    # Pair q (ic in {0,1}: q=b; ic in {2,3}: q=nb+b) holds
                    # tile (ic&1) at XB8[:, q, ic&1, :].
                    base = 0 if ic < 2 else nb
                    dst = xb8[:, base:base + nb, ic & 1, :]
                    src_ = p1[:, 0:nb * 128]
                    if eng is nc.vector:
                        eng.tensor_copy(dst, src_)
                    else:
                        eng.copy(dst, src_)
                if nfull:
                    dst = xb32[:, ic, :]
                    src_ = p1[:, nb * 128:1152]
                    if eng is nc.vector:
                        eng.tensor_copy(dst, src_)
                    else:
                        eng.copy(dst, src_)

        def ph2(g, xb8, xb32, stile):
            p2 = ps2.tile([128, J], F32, tag="p2")
            k, klast = 0, 2 * nb + 4 * nfull - 1
            for q in range(2 * nb):
                nc.tensor.matmul(
                    p2[:],
                    xb8[:, q, :, :],
                    w8t[q][:],
                    start=(k == 0), stop=(k == klast),
                    perf_mode=DR,
                )
                k += 1
            for ic in range(4):
                for bi in range(nfull):
                    nc.tensor.matmul(
                        p2[:],
                        xb32[:, ic, bi * 128:(bi + 1) * 128],
                        w2t[bi, ic][:],
                        start=(k == 0), stop=(k == klast),
                    )
                    k += 1
            ot = osb.tile([128, J], F32, tag="osb")
            if nb:
                nc.scalar.add(ot[:], p2[:], stile[:])
            else:
                nc.scalar.copy(ot[:], p2[:])
            nc.sync.dma_start(out=out_d[g], in_=ot[:])

        # Software pipelining: phase-1 of group g is emitted in two halves
        # around phase-2 of group g-1, so phase-1's psum-slot waits overlap
        # phase-2's long matmul stream instead of stalling PE.
        pending = None
        for g in [g for _ in range(repeats) for g in range(groups)]:
            xt = xtp.tile([128, C], mmdt, tag="xt")
            nc.sync.dma_start(out=xt[:], in_=xT_d[g])
            xb8 = xbp.tile([128, 2 * nb, 2, 128], FP8, tag="xb8", name="xb8") if nb else None
            xb32 = (xbp.tile([128, 4, nfull * 128], mmdt, tag="xb32", name="xb32")
                    if nfull else None)
            stile = None
            if nb:
                stile = ssb.tile([128, 1], F32, tag="s", name="stile")
                nc.sync.dma_start(out=stile[:], in_=s_d[g])
            ph1_half(g, xt, (0, 1), xb8, xb32)
            if pending is not None:
                ph2(*pending)
            ph1_half(g, xt, (2, 3), xb8, xb32)
            pending = (g, xb8, xb32, stile)
        if pending is not None:
            ph2(*pending)

    nc.compile()
    return nc


def round_fp32r(a: np.ndarray) -> np.ndarray:
    """Round fp32 to the PE's fp32r format (e8m11): RNE to 11 mantissa bits."""
    u = np.ascontiguousarray(a, np.float32).view(np.uint32)
    lsb = (u >> 12) & 1
    u = ((u + 0x7FF + lsb) & np.uint32(0xFFFFF000)).astype(np.uint32)
    return u.view(np.float32)


def blade_order(blade: np.ndarray, nb: int) -> np.ndarray:
    """Reorder b so the nb lowest-signal-energy blades (least fp8 error) come
    first."""
    energy = np.square(np.asarray(blade, np.float64)).sum(axis=(1, 2))
    order = np.argsort(energy)          # ascending: fp8 ones first
    head = np.sort(order[:nb])
    tail = np.sort(order[nb:])
    return np.concatenate([head, tail])


def prep_inputs(x: np.ndarray, weights: np.ndarray, blade: np.ndarray,
                nb: int = NB_FP8):
    """Host-side layout prep. Returns dict of per-core-shared tensors plus the
    per-core xT slices stacked as [GROUPS, 128, C]."""
    nfull = BL - nb
    order = blade_order(blade, nb)
    bl = round_fp32r(np.asarray(blade, np.float32)[order])
    w = np.asarray(weights, np.float32)[:, :, order]

    x = np.ascontiguousarray(x, dtype=np.float32)
    xT = x.reshape(GROUPS, 8, C, MV).transpose(0, 1, 3, 2).reshape(GROUPS, 128, C)
    xT = round_fp32r(np.ascontiguousarray(xT))

    RB = np.zeros((128, 1280), np.float32)
    for b in range(BL):
        for dpt in range(8):
            RB[dpt * 16:(dpt + 1) * 16,
               b * 128 + dpt * 16: b * 128 + (dpt + 1) * 16] = bl[b]

    W2full = np.ascontiguousarray(w.transpose(2, 1, 0))        # [b, i, j]
    out = {"RB": RB, "xT": xT}
    if nb:
        w8 = np.empty((2 * nb, 128, 2, J), E4M3)
        wshift = (W2full[:nb] - 0.5).astype(np.float32)
        for q in range(2 * nb):
            b, ic0 = (q, 0) if q < nb else (q - nb, 2)
            w8[q, :, 0, :] = wshift[b, ic0 * 128:(ic0 + 1) * 128, :].astype(E4M3)
            w8[q, :, 1, :] = wshift[b, (ic0 + 1) * 128:(ic0 + 2) * 128, :].astype(E4M3)
        out["W8"] = w8
        bs_small = 0.5 * bl[:nb].sum(axis=0).astype(np.float64)   # [x, y]
        sxh = xT.astype(np.float64).sum(axis=2).reshape(GROUPS, 8, MV)
        SG = np.einsum("gdx,xy->gdy", sxh, bs_small).astype(np.float32)
        out["SG"] = np.ascontiguousarray(SG.reshape(GROUPS, 128, 1))
    if nfull:
        W2 = round_fp32r(W2full[nb:]).reshape(nfull, 4, 128, J)
        out["W2"] = np.ascontiguousarray(W2)
    return out


def unprep_output(outT_all: np.ndarray) -> np.ndarray:
    """outT_all [GROUPS,128,J] -> out [BATCH,NPTS,J,MV]."""
    return np.ascontiguousarray(
        outT_all.reshape(GROUPS, 8, MV, J).transpose(0, 1, 3, 2)
        .reshape(BATCH, NPTS, J, MV)
    )


_NC_CACHE = {}


def _get_program(nb: int = NB_FP8):
    key = (GROUPS_PER_CORE, nb)
    if key not in _NC_CACHE:
        _NC_CACHE[key] = build_program(GROUPS_PER_CORE, nb)
    return _NC_CACHE[key]


def make_in_maps(tensors: dict, gpc: int = GROUPS_PER_CORE,
                 groups: int | None = None) -> list[dict]:
    """Slice the per-group tensors (xT, SG) per core; share the rest."""
    g = gpc if groups is None else groups
    shared = {k: v for k, v in tensors.items() if k not in ("xT", "SG")}
    maps = []
    for c in range(N_CORES):
        m = {"xT": tensors["xT"][c * gpc:c * gpc + g], **shared}
        if "SG" in tensors:
            m["SG"] = tensors["SG"][c * gpc:c * gpc + g]
        maps.append(m)
    return maps


def kernel(x: np.ndarray, weights: np.ndarray, blade: np.ndarray) -> np.ndarray:
    tensors = prep_inputs(x, weights, blade, NB_FP8)
    nc = _get_program(NB_FP8)
    in_maps = make_in_maps(tensors)
    try:
        res = run_bass_kernel_spmd(nc, in_maps, list(range(N_CORES))).results
    except Exception:
        # Transient NRT/axon faults have been observed across rapid successive
        # sessions; retry once.
        import time as _time
        _time.sleep(10)
        res = run_bass_kernel_spmd(nc, in_maps, list(range(N_CORES))).results
    outT_all = np.concatenate([res[c]["outT"] for c in range(N_CORES)], axis=0)
    return unprep_output(outT_all)
